# revision 4
# baseline (speedup 1.0000x reference)
"""GAT layer (N=8192, IN_F=512, OUT_F=128) on 8 TRN2 NeuronCores.

Sharding: rows of the attention matrix are split across cores (1024 rows
each).  Each core receives its row-slab of M and adj pre-transposed on the
host to [8192, 1024] so the attention weights are computed directly in
[j, i] orientation (contraction index j on partitions), which the final
attention @ h matmul requires.  h is computed on every core from a
replicated input.T.

Per-core pipeline:
  A) Wa = W @ [a_self | a_neighs] (PE);  s-row for own rows via fp32r
     matmul; partition-broadcast of s via a K=1 outer-product matmul.
  B) h = input @ W in fp32r with rhs [W | Wa_n | 0-pad to 256]; the
     attn_neighs scores fall out as psum column 128 per n-block; h is
     cast to bf16 into h_aug ([h | 1] per block; ones column makes the
     main matmul emit softmax row-sums for free).
  C) For each j-block: Z = (s_i + n_j) * M^T (one fused DVE op);
     leaky_relu via Prelu(alpha=0.2) on ACT (note leaky(x*M) = M*leaky(x)
     is NOT needed - we apply leaky after the multiply, exactly as the
     reference); Exp on ACT; mask-multiply by adj^T into bf16; 8
     accumulating bf16 matmuls into 8 PSUM tiles [128, 129].
  D) Row-sum reciprocals (DVE), normalize, ELU, DMA out.

Softmax skips the max-subtraction: logits are bounded (~+-30) so exp is
safe in fp32, and the result is mathematically identical.
"""

import os
import numpy as np

_N = 8192      # nodes
_K = 512       # in features
_F = 128       # out features
_C = 8         # cores
_R = _N // _C  # rows per core (1024)
_KB = _K // 128   # 4  k-blocks
_NB = _N // 128   # 64 j/n-blocks
_IB = _R // 128   # 8  i-blocks per core

_NC_CACHE = {}
LAST_RESULTS = None


def _build_nc():
    from contextlib import ExitStack
    import concourse.bacc as bacc
    import concourse.tile as tile
    from concourse import mybir

    F32 = mybir.dt.float32
    F32R = mybir.dt.float32r
    BF16 = mybir.dt.bfloat16
    A = mybir.ActivationFunctionType
    Op = mybir.AluOpType

    nc = bacc.Bacc("TRN2", target_bir_lowering=False, debug=False,
                   num_devices=_C)

    xT = nc.dram_tensor("xT", (_K, _N), F32R, kind="ExternalInput").ap()
    xTo = nc.dram_tensor("xTo", (_K, _R), F32R, kind="ExternalInput").ap()
    mT = nc.dram_tensor("mT", (_N, _R), F32, kind="ExternalInput").ap()
    aT = nc.dram_tensor("aT", (_N, _R), F32, kind="ExternalInput").ap()
    Wd = nc.dram_tensor("Wd", (_K, _F), F32R, kind="ExternalInput").ap()
    WTd = nc.dram_tensor("WTd", (_F, _K), F32, kind="ExternalInput").ap()
    abd = nc.dram_tensor("abd", (_F, 2), F32, kind="ExternalInput").ap()
    outd = nc.dram_tensor("out", (_R, _F), F32, kind="ExternalOutput").ap()

    with tile.TileContext(nc) as tc, ExitStack() as ctx:
        persist = ctx.enter_context(tc.tile_pool(name="persist", bufs=1))
        h_aug = persist.tile([128, _NB * 129], BF16)   # [h | 1] per j-block
        n_all = persist.tile([128, _NB], F32)          # attn_neighs per j
        s_bc = persist.tile([128, _R], F32)            # attn_self bcast
        params = ctx.enter_context(tc.tile_pool(name="params", bufs=1))
        w_rhs = params.tile([128, _KB, 256], F32R)     # [W | Wa_n | 0]
        wa = params.tile([128, _KB, 2], F32R)          # W @ [a_self|a_neighs]

        nc.vector.memset(h_aug[:], 1.0)
        nc.vector.memset(w_rhs[:].bitcast(mybir.dt.uint32), 0)

        # ---- Phase A: params, Wa, s-row, s broadcast -------------------
        with tc.tile_pool(name="pha", bufs=1) as pa, \
             tc.tile_pool(name="pps", bufs=2, space="PSUM") as pp:
            wt_sb = pa.tile([_F, _K], F32)
            nc.sync.dma_start(wt_sb[:], WTd)
            ab_sb = pa.tile([_F, 2], F32)
            nc.sync.dma_start(ab_sb[:], abd)
            for k in range(_KB):
                nc.sync.dma_start(w_rhs[:, k, 0:_F], Wd[k * 128:(k + 1) * 128, :])
            for k in range(_KB):
                pwa = pp.tile([128, 2], F32)
                nc.tensor.matmul(pwa[:], wt_sb[:, k * 128:(k + 1) * 128],
                                 ab_sb[:], start=True, stop=True)
                nc.vector.tensor_copy(wa[:, k, :], pwa[:])
                nc.vector.tensor_copy(w_rhs[:, k, _F:_F + 1], pwa[:, 1:2])

            xo = pa.tile([128, _KB, _R], F32R)
            for k in range(_KB):
                nc.sync.dma_start(xo[:, k, :], xTo[k * 128:(k + 1) * 128, :])
            s_row = pa.tile([1, _R], F32)
            for ch in range(_R // 512):
                pss = pp.tile([1, 512], F32)
                for k in range(_KB):
                    nc.tensor.matmul(pss[:], wa[:, k, 0:1],
                                     xo[:, k, ch * 512:(ch + 1) * 512],
                                     start=(k == 0), stop=(k == _KB - 1))
                nc.vector.tensor_copy(s_row[:, ch * 512:(ch + 1) * 512], pss[:])
            ones1 = pa.tile([1, 128], F32)
            nc.vector.memset(ones1[:], 1.0)
            for ch in range(_R // 512):
                psb = pp.tile([128, 512], F32)
                nc.tensor.matmul(psb[:], ones1[:],
                                 s_row[:, ch * 512:(ch + 1) * 512],
                                 start=True, stop=True)
                nc.vector.tensor_copy(s_bc[:, ch * 512:(ch + 1) * 512], psb[:])

        # ---- Phase B: h = input @ [W | Wa_n | 0] -----------------------
        with tc.tile_pool(name="xts", bufs=3) as px, \
             tc.tile_pool(name="phps", bufs=2, space="PSUM") as ph:
            for ch in range(_N // 512):
                xt_t = px.tile([128, _KB, 512], F32R)
                for k in range(_KB):
                    nc.sync.dma_start(
                        xt_t[:, k, :],
                        xT[k * 128:(k + 1) * 128, ch * 512:(ch + 1) * 512])
                for nl in range(4):
                    nb = ch * 4 + nl
                    psh = ph.tile([128, 256], F32)
                    for k in range(_KB):
                        nc.tensor.matmul(psh[:],
                                         xt_t[:, k, nl * 128:(nl + 1) * 128],
                                         w_rhs[:, k, :],
                                         start=(k == 0), stop=(k == _KB - 1))
                    nc.vector.tensor_copy(h_aug[:, nb * 129:nb * 129 + _F],
                                          psh[:, 0:_F])
                    nc.scalar.copy(n_all[:, nb:nb + 1], psh[:, _F:_F + 1])

        # ---- Phase C: attention weights + accumulating matmul ----------
        mainp = ctx.enter_context(tc.tile_pool(name="mts", bufs=10))
        zp = ctx.enter_context(tc.tile_pool(name="zp", bufs=2))
        pso = ctx.enter_context(tc.tile_pool(name="pso", bufs=1, space="PSUM"))
        psum_o = [pso.tile([128, 129], F32, name=f"po{i}", tag=f"po{i}")
                  for i in range(_IB)]
        for jb in range(_NB):
            mt_t = mainp.tile([128, _R], F32, tag="mt")
            at_t = mainp.tile([128, _R], F32, tag="at")
            nc.sync.dma_start(mt_t[:], mT[jb * 128:(jb + 1) * 128, :])
            nc.sync.dma_start(at_t[:], aT[jb * 128:(jb + 1) * 128, :])
            z = zp.tile([128, _R], F32, tag="z")
            nc.vector.scalar_tensor_tensor(z[:], s_bc[:], n_all[:, jb:jb + 1],
                                           mt_t[:], op0=Op.add, op1=Op.mult)
            lk = zp.tile([128, _R], F32, tag="lk")
            nc.scalar.activation(lk[:], z[:], A.Prelu, bias=0.0, scale=1.0,
                                 alpha=0.2)
            ex = zp.tile([128, _R], F32, tag="ex")
            nc.scalar.activation(ex[:], lk[:], A.Exp)
            wb = zp.tile([128, _R], BF16, tag="wb")
            nc.vector.tensor_mul(wb[:], ex[:], at_t[:])
            for ib in range(_IB):
                nc.tensor.matmul(psum_o[ib][:],
                                 wb[:, ib * 128:(ib + 1) * 128],
                                 h_aug[:, jb * 129:(jb + 1) * 129],
                                 start=(jb == 0), stop=(jb == _NB - 1))

        # ---- Phase D: normalize + ELU + store --------------------------
        finp = ctx.enter_context(tc.tile_pool(name="finp", bufs=1))
        rs = finp.tile([128, _IB], F32)
        ri = finp.tile([128, _IB], F32)
        for ib in range(_IB):
            nc.vector.tensor_copy(rs[:, ib:ib + 1], psum_o[ib][:, _F:_F + 1])
        nc.vector.reciprocal(ri[:], rs[:])
        fin2 = ctx.enter_context(tc.tile_pool(name="fin2", bufs=2))
        for ib in range(_IB):
            hp = fin2.tile([128, _F], F32, tag="hp")
            nc.vector.tensor_scalar(hp[:], psum_o[ib][:, 0:_F],
                                    ri[:, ib:ib + 1], None, op0=Op.mult)
            ex2 = fin2.tile([128, _F], F32, tag="ex2")
            nc.scalar.activation(ex2[:], hp[:], A.Exp)
            em = fin2.tile([128, _F], F32, tag="em")
            nc.vector.tensor_scalar(em[:], ex2[:], -1.0, 0.0,
                                    op0=Op.add, op1=Op.min)
            rl = fin2.tile([128, _F], F32, tag="rl")
            nc.vector.tensor_scalar(rl[:], hp[:], 0.0, None, op0=Op.max)
            ot = fin2.tile([128, _F], F32, tag="ot")
            nc.vector.tensor_add(ot[:], em[:], rl[:])
            nc.sync.dma_start(outd[ib * 128:(ib + 1) * 128, :], ot[:])

    nc.compile()
    return nc


def kernel(input, adj, M, W, a_self, a_neighs):
    global LAST_RESULTS
    from concourse.bass_utils import run_bass_kernel_spmd

    if "nc" not in _NC_CACHE:
        _NC_CACHE["nc"] = _build_nc()
    nc = _NC_CACHE["nc"]

    inp = np.ascontiguousarray(np.asarray(input, dtype=np.float32))
    adj_ = np.asarray(adj, dtype=np.float32)
    M_ = np.asarray(M, dtype=np.float32)
    W_ = np.ascontiguousarray(np.asarray(W, dtype=np.float32))
    a_s = np.asarray(a_self, dtype=np.float32).reshape(_F, 1)
    a_n = np.asarray(a_neighs, dtype=np.float32).reshape(_F, 1)

    xT_full = np.ascontiguousarray(inp.T)           # [512, 8192]
    WT = np.ascontiguousarray(W_.T)                 # [128, 512]
    ab = np.ascontiguousarray(np.concatenate([a_s, a_n], axis=1))  # [128, 2]

    in_maps = []
    for c in range(_C):
        rows = slice(c * _R, (c + 1) * _R)
        in_maps.append({
            "xT": xT_full,
            "xTo": np.ascontiguousarray(inp[rows].T),
            "mT": np.ascontiguousarray(M_[rows].T),
            "aT": np.ascontiguousarray(adj_[rows].T),
            "Wd": W_,
            "WTd": WT,
            "abd": ab,
        })

    res = run_bass_kernel_spmd(nc, in_maps, core_ids=list(range(_C)),
                               trace=bool(os.environ.get("BASS_TRACE")))
    LAST_RESULTS = res
    out = np.concatenate([res.results[c]["out"] for c in range(_C)], axis=0)
    return np.ascontiguousarray(out.astype(np.float32))


# revision 14
# speedup vs baseline: 1.2637x; 1.2637x over previous
"""GAT layer (N=8192, IN_F=512, OUT_F=128) on 8 TRN2 NeuronCores.

Sharding: rows of the attention matrix are split across cores (1024 rows
each).  Each core receives its row-slab of M and adj pre-transposed on the
host to [8192, 1024] so the attention weights are computed directly in
[j, i] orientation (contraction index j on partitions), which the final
attention @ h matmul requires.  h is computed on every core from a
replicated input.T.

Per-core pipeline:
  A) Wa = W @ [a_self | a_neighs] (PE);  s-row for own rows via fp32r
     matmul; partition-broadcast of s via a K=1 outer-product matmul.
  B) h = input @ W in fp32r with rhs [W | Wa_n | 0-pad to 256]; the
     attn_neighs scores fall out as psum column 128 per n-block; h is
     cast to bf16 into h_aug ([h | 1] per block; ones column makes the
     main matmul emit softmax row-sums for free).
  C) For each j-block: Z = (s_i + n_j) * M^T (one fused DVE op);
     leaky_relu via Prelu(alpha=0.2) on ACT (note leaky(x*M) = M*leaky(x)
     is NOT needed - we apply leaky after the multiply, exactly as the
     reference); Exp on ACT; mask-multiply by adj^T into bf16; 8
     accumulating bf16 matmuls into 8 PSUM tiles [128, 129].
  D) Row-sum reciprocals (DVE), normalize, ELU, DMA out.

Softmax skips the max-subtraction: logits are bounded (~+-30) so exp is
safe in fp32, and the result is mathematically identical.
"""

import os
import ml_dtypes
import numpy as np

_N = 8192      # nodes
_K = 512       # in features
_F = 128       # out features
_C = 8         # cores
_R = _N // _C  # rows per core (1024)
_KB = _K // 128   # 4  k-blocks
_NB = _N // 128   # 64 j/n-blocks
_IB = _R // 128   # 8  i-blocks per core

_NC_CACHE = {}
LAST_RESULTS = None


def _build_nc():
    from contextlib import ExitStack
    import concourse.bacc as bacc
    import concourse.tile as tile
    from concourse import mybir

    F32 = mybir.dt.float32
    F32R = mybir.dt.float32r
    BF16 = mybir.dt.bfloat16
    A = mybir.ActivationFunctionType
    Op = mybir.AluOpType

    nc = bacc.Bacc("TRN2", target_bir_lowering=False, debug=False,
                   num_devices=_C)

    xT = nc.dram_tensor("xT", (_K, _N), F32R, kind="ExternalInput").ap()
    xTo = nc.dram_tensor("xTo", (_K, _R), F32R, kind="ExternalInput").ap()
    mT = nc.dram_tensor("mT", (_N, _R), F32, kind="ExternalInput").ap()
    aT = nc.dram_tensor("aT", (_N, _R), BF16, kind="ExternalInput").ap()
    Wd = nc.dram_tensor("Wd", (_K, _F), F32R, kind="ExternalInput").ap()
    WTd = nc.dram_tensor("WTd", (_F, _K), F32, kind="ExternalInput").ap()
    abd = nc.dram_tensor("abd", (_F, 2), F32, kind="ExternalInput").ap()
    outd = nc.dram_tensor("out", (_R, _F), F32, kind="ExternalOutput").ap()

    _G = 8       # j-block groups (dependency granularity for B->C overlap)
    _JPG = _NB // _G   # 8 j-blocks per group

    with tile.TileContext(nc) as tc, ExitStack() as ctx:
        persist = ctx.enter_context(tc.tile_pool(name="persist", bufs=1))
        # [h | 1] per j-block, split in _G group tiles so phase C can start
        # on group g as soon as phase B finished writing it.
        h_aug = [persist.tile([128, _JPG * 129], BF16, name=f"haug{g}",
                              tag=f"haug{g}") for g in range(_G)]
        n_all = [persist.tile([128, _JPG], F32, name=f"nall{g}",
                              tag=f"nall{g}") for g in range(_G)]
        s_bc = persist.tile([128, _R], F32)            # attn_self bcast
        params = ctx.enter_context(tc.tile_pool(name="params", bufs=1))
        w_rhs = params.tile([128, _KB, 256], F32R)     # [W | Wa_n | 0]
        wa = params.tile([128, _KB, 2], F32R)          # W @ [a_self|a_neighs]

        for g in range(_G):
            nc.vector.memset(h_aug[g][:], 1.0)
        nc.vector.memset(w_rhs[:].bitcast(mybir.dt.uint32), 0)

        # ---- Phase A: params, Wa, s-row, s broadcast -------------------
        pa = ctx.enter_context(tc.tile_pool(name="pha", bufs=1))
        with tc.tile_pool(name="pps", bufs=2, space="PSUM") as pp:
            wt_sb = pa.tile([_F, _K], F32)
            nc.sync.dma_start(wt_sb[:], WTd)
            ab_sb = pa.tile([_F, 2], F32)
            nc.sync.dma_start(ab_sb[:], abd)
            for k in range(_KB):
                nc.sync.dma_start(w_rhs[:, k, 0:_F], Wd[k * 128:(k + 1) * 128, :])
            for k in range(_KB):
                pwa = pp.tile([128, 2], F32)
                nc.tensor.matmul(pwa[:], wt_sb[:, k * 128:(k + 1) * 128],
                                 ab_sb[:], start=True, stop=True)
                nc.vector.tensor_copy(wa[:, k, :], pwa[:])
                nc.vector.tensor_copy(w_rhs[:, k, _F:_F + 1], pwa[:, 1:2])

            xo = pa.tile([128, _KB, _R], F32R)
            for k in range(_KB):
                nc.sync.dma_start(xo[:, k, :], xTo[k * 128:(k + 1) * 128, :])
            s_row = pa.tile([1, _R], F32)
            for ch in range(_R // 512):
                pss = pp.tile([1, 512], F32)
                for k in range(_KB):
                    nc.tensor.matmul(pss[:], wa[:, k, 0:1],
                                     xo[:, k, ch * 512:(ch + 1) * 512],
                                     start=(k == 0), stop=(k == _KB - 1))
                nc.vector.tensor_copy(s_row[:, ch * 512:(ch + 1) * 512], pss[:])
            ones1 = pa.tile([1, 128], F32)
            nc.vector.memset(ones1[:], 1.0)
            for ch in range(_R // 512):
                psb = pp.tile([128, 512], F32)
                nc.tensor.matmul(psb[:], ones1[:],
                                 s_row[:, ch * 512:(ch + 1) * 512],
                                 start=True, stop=True)
                nc.vector.tensor_copy(s_bc[:, ch * 512:(ch + 1) * 512], psb[:])

        # ---- Phase B: h = input @ [W | Wa_n | 0] -----------------------
        px = ctx.enter_context(tc.tile_pool(name="xts", bufs=3))
        ph = ctx.enter_context(tc.tile_pool(name="phps", bufs=2, space="PSUM"))
        for ch in range(_N // 512):
            xt_t = px.tile([128, _KB, 512], F32R)
            for k in range(_KB):
                nc.sync.dma_start(
                    xt_t[:, k, :],
                    xT[k * 128:(k + 1) * 128, ch * 512:(ch + 1) * 512])
            for nl in range(4):
                nb = ch * 4 + nl
                g, bl = nb // _JPG, nb % _JPG
                psh = ph.tile([128, 256], F32)
                for k in range(_KB):
                    nc.tensor.matmul(psh[:],
                                     xt_t[:, k, nl * 128:(nl + 1) * 128],
                                     w_rhs[:, k, :],
                                     start=(k == 0), stop=(k == _KB - 1))
                nc.vector.tensor_copy(h_aug[g][:, bl * 129:bl * 129 + _F],
                                      psh[:, 0:_F])
                nc.scalar.copy(n_all[g][:, bl:bl + 1], psh[:, _F:_F + 1])

        # ---- Phase C: attention weights + accumulating matmul ----------
        mainp = ctx.enter_context(tc.tile_pool(name="mts", bufs=10))
        zp = ctx.enter_context(tc.tile_pool(name="zp", bufs=2))
        pso = ctx.enter_context(tc.tile_pool(name="pso", bufs=1, space="PSUM"))
        # two [128, 129] accumulation regions packed per PSUM bank
        psum_o = [pso.tile([128, 2 * 129], F32, name=f"po{i}", tag=f"po{i}")
                  for i in range(_IB // 2)]

        def _po(ib):
            return psum_o[ib // 2][:, (ib % 2) * 129:(ib % 2) * 129 + 129]

        # Zero-init each packed bank with one K=1 outer-product matmul
        # (start=True zeroes the whole 2KB zero-region, so per-region
        # start flags would wipe the sibling region's accumulation).
        zrow = params.tile([1, 2 * 129], BF16)
        ones1b = params.tile([1, 128], BF16)
        nc.vector.memset(zrow[:], 0.0)
        nc.vector.memset(ones1b[:], 1.0)
        for bank in range(_IB // 2):
            nc.tensor.matmul(psum_o[bank][:], ones1b[:], zrow[:],
                             start=True, stop=False, skip_group_check=True)

        for jb in range(_NB):
            g, bl = jb // _JPG, jb % _JPG
            mt_t = mainp.tile([128, _R], F32, tag="mt")
            at_t = mainp.tile([128, _R], BF16, tag="at")
            nc.sync.dma_start(mt_t[:], mT[jb * 128:(jb + 1) * 128, :])
            nc.sync.dma_start(at_t[:], aT[jb * 128:(jb + 1) * 128, :])
            z = zp.tile([128, _R], F32, tag="z")
            nc.vector.scalar_tensor_tensor(z[:], s_bc[:],
                                           n_all[g][:, bl:bl + 1],
                                           mt_t[:], op0=Op.add, op1=Op.mult)
            lk = zp.tile([128, _R], F32, tag="lk")
            nc.scalar.activation(lk[:], z[:], A.Prelu, bias=0.0, scale=1.0,
                                 alpha=0.2)
            ex = zp.tile([128, _R], BF16, tag="ex")
            nc.scalar.activation(ex[:], lk[:], A.Exp)
            wb = zp.tile([128, _R], BF16, tag="wb")
            nc.vector.tensor_mul(wb[:], ex[:], at_t[:])
            for ib in range(_IB):
                nc.tensor.matmul(_po(ib),
                                 wb[:, ib * 128:(ib + 1) * 128],
                                 h_aug[g][:, bl * 129:(bl + 1) * 129],
                                 start=False, stop=(jb == _NB - 1),
                                 skip_group_check=True)

        # ---- Phase D: normalize + ELU + store --------------------------
        finp = ctx.enter_context(tc.tile_pool(name="finp", bufs=1))
        rs = finp.tile([128, _IB], F32)
        ri = finp.tile([128, _IB], F32)
        for ib in range(_IB):
            nc.vector.tensor_copy(rs[:, ib:ib + 1], _po(ib)[:, _F:_F + 1])
        nc.vector.reciprocal(ri[:], rs[:])
        fin2 = ctx.enter_context(tc.tile_pool(name="fin2", bufs=2))
        for ib in range(_IB):
            hp = fin2.tile([128, _F], F32, tag="hp")
            nc.vector.tensor_scalar(hp[:], _po(ib)[:, 0:_F],
                                    ri[:, ib:ib + 1], None, op0=Op.mult)
            ex2 = fin2.tile([128, _F], F32, tag="ex2")
            nc.scalar.activation(ex2[:], hp[:], A.Exp)
            em = fin2.tile([128, _F], F32, tag="em")
            nc.vector.tensor_scalar(em[:], ex2[:], -1.0, 0.0,
                                    op0=Op.add, op1=Op.min)
            rl = fin2.tile([128, _F], F32, tag="rl")
            nc.vector.tensor_scalar(rl[:], hp[:], 0.0, None, op0=Op.max)
            ot = fin2.tile([128, _F], F32, tag="ot")
            nc.vector.tensor_add(ot[:], em[:], rl[:])
            nc.sync.dma_start(outd[ib * 128:(ib + 1) * 128, :], ot[:])

    nc.compile()
    return nc


def kernel(input, adj, M, W, a_self, a_neighs):
    global LAST_RESULTS
    from concourse.bass_utils import run_bass_kernel_spmd

    if "nc" not in _NC_CACHE:
        _NC_CACHE["nc"] = _build_nc()
    nc = _NC_CACHE["nc"]

    inp = np.ascontiguousarray(np.asarray(input, dtype=np.float32))
    adj_ = np.asarray(adj, dtype=np.float32)
    M_ = np.asarray(M, dtype=np.float32)
    W_ = np.ascontiguousarray(np.asarray(W, dtype=np.float32))
    a_s = np.asarray(a_self, dtype=np.float32).reshape(_F, 1)
    a_n = np.asarray(a_neighs, dtype=np.float32).reshape(_F, 1)

    xT_full = np.ascontiguousarray(inp.T)           # [512, 8192]
    WT = np.ascontiguousarray(W_.T)                 # [128, 512]
    ab = np.ascontiguousarray(np.concatenate([a_s, a_n], axis=1))  # [128, 2]

    in_maps = []
    for c in range(_C):
        rows = slice(c * _R, (c + 1) * _R)
        in_maps.append({
            "xT": xT_full,
            "xTo": np.ascontiguousarray(inp[rows].T),
            "mT": np.ascontiguousarray(M_[rows].T),
            "aT": np.ascontiguousarray(adj_[rows].T.astype(ml_dtypes.bfloat16)),
            "Wd": W_,
            "WTd": WT,
            "abd": ab,
        })

    res = run_bass_kernel_spmd(nc, in_maps, core_ids=list(range(_C)),
                               trace=bool(os.environ.get("BASS_TRACE")))
    LAST_RESULTS = res
    out = np.concatenate([res.results[c]["out"] for c in range(_C)], axis=0)
    return np.ascontiguousarray(out.astype(np.float32))
